# revision 35
# baseline (speedup 1.0000x reference)
"""Multi-head attention (B=4, S=2048, E=768, H=12, Dh=64) on 8 TRN2 NeuronCores.

Sharding: batch x head-group tensor parallel. Core c handles batch b = c//2 and
head group g = c%2 (6 heads each). Each core computes its heads' Q/K/V
projections, full attention over the 2048-token sequence, and a partial
out-projection over its 384 concat-features. The host sums the two partials per
batch and adds the output bias.

Numerics/engine layout (what makes this fast):
 - Q/K projections and the score matmuls run in fp8e4m3 with the DoubleRow
   perf mode (0.5 PE cycles per output column, 2x contraction per matmul).
   Host packs x^T and the QK weights in a [128, pairs=2, .] chunk-pair layout;
   weights are pre-scaled by 16 so their 0.02-sigma values sit in e4m3's
   normal range (the 16*16 factor is folded into the softmax exp scale, which
   stays exactly 2^-11). The key bias is dropped entirely: adding bk shifts
   every logit of a query by a per-query constant, which softmax cancels.
 - Scores contract over dh=64 only, so their DoubleRow pair slot 1 points at
   a zeroed region of the qT/kT tiles (w1=0 makes slot 1 a no-op); the cost
   model charges by output columns, so this halves score cost versus bf16.
 - PV, the V projection and the out-projection stay bf16: e4m3's 6% rounding
   on p/v/wo produces error tails correlated across a head's features that
   blow past the 2e-2 gate (measured 1.7e-2+ in simulation), while scores-fp8
   lands at ~1e-2 total.
 - exp is the second-largest serial cost (192 exps of [128,1024] at ~1.04us
   on ACT = 200us > PE). Pool cannot read PSUM and DVE's ALU has no pow, so
   every 4th score tile is staged: DVE copies it psum->sbuf as bf16 (the
   ~0.4% logit rounding is harmless) and the Pool engine computes
   pow(e^(2^-11), s) from SBUF against a persistent f32 base tile (bf16
   cannot represent e^(2^-11)). Staged tiles are {0,6,8,11} per half-head:
   tile 0's scores are pre-issued during the previous head so the slow
   Pool chain gets a long lead, and nothing in the last group of 4 is
   staged (a staged stop-tile would put the Pool chain on the
   head-boundary critical path). That leaves ACT ~151us, DVE ~130us
   (staging + reciprocal + normalize + other psum->sbuf copies), Pool
   ~92us, under the ~167us PE stream.
 - The PSUM layout is 2 score tiles [128,1024] + context [128,1024] +
   3 ring slots... scores ring is 3 deep ([128,1024] x3 + ct = 16KB
   exactly), with projection / out-projection groups BORROWING ring slots
   (no separate proj psum): depth 3 is what lets the ACT exp stream run
   back-to-back instead of locking step with PE's slot recycling.
 - PV accumulation order is permuted (staged tiles accumulate last in
   their group of 4) - it is a sum over key tiles, so order is free.
 - All deferrable PE work (next feature-tile QK projections, early
   out-projection chunks) drips one ~0.5us thunk per odd pipeline step
   inside the head loops; multi-us PE-only bursts would starve ACT. The
   final out-projection flush alternates its psum-drain copies ACT/DVE.
 - V is token-major, each head augmented with 64 ones columns so the PV
   matmul emits the softmax denominator replicated on psum partitions 64-127
   (normalization is one DVE reciprocal + one multiply-cast); scores are
   computed transposed (S^T tiles [128 keys, queries]) and exponentiated
   straight out of PSUM (no max-subtraction: logits are ~N(0, 0.3)).
"""

import math
import os
import sys
from contextlib import ExitStack

import numpy as np

for _p in ("/opt/trn_rl_repo", "/root/.axon_site/_ro/trn_rl_repo"):
    if os.path.isdir(_p) and _p not in sys.path:
        sys.path.append(_p)

# NTFF tracing hooks (antenv.axon_hooks) don't exist in this container;
# make sure an ambient BASS_TRACE can't route execution into that path.
os.environ["BASS_NEVER_TRACE"] = "1"

import ml_dtypes  # noqa: E402

import concourse.bass as bass  # noqa: E402
import concourse.tile as tile  # noqa: E402
from concourse import bacc, mybir  # noqa: E402
from concourse.bass_utils import run_bass_kernel_spmd  # noqa: E402

BF16 = mybir.dt.bfloat16
F32 = mybir.dt.float32
F8 = mybir.dt.float8e4
NP_BF16 = ml_dtypes.bfloat16
NP_F8 = ml_dtypes.float8_e4m3
DRMODE = mybir.MatmulPerfMode.DoubleRow

B, S, E, H, DH = 4, 2048, 768, 12, 64
N_CORES = 8
G = H // 2  # heads per core (6)
WS = 16.0  # fp8 weight pre-scale; folded into the exp scale


def build_nc(T=S, EMB=E, NH=G, dh=DH, OUT=E, trace_label="", staged=(0, 6, 8, 11)):
    """Emit the per-core Bass/Tile program. All cores run this same program.

    T: sequence length; EMB: model dim; NH: heads on this core (even);
    dh: head dim (64); OUT: out-projection output width.
    """
    assert T % 128 == 0 and EMB % 128 == 0 and dh == 64 and NH % 2 == 0
    FEAT = NH * dh
    assert FEAT % 128 == 0
    EC = EMB // 128  # contraction chunks for projections
    EP = EC // 2  # fp8 chunk-pairs for the QK projections
    assert EC % 2 == 0
    TT = T // 128  # token tiles
    FT = FEAT // 128  # feature tiles (head pairs)
    SCH = min(512, T)  # matmul moving free-dim chunk
    NSCH = T // SCH
    T2 = max(128, T // 2)  # attention query-half width (2 PSUM banks)
    NSH = T // T2  # query halves per head
    SCH2 = min(512, T2)
    NSCH2 = T2 // SCH2
    _ock = OUT // 2 if 128 < OUT <= 1024 and OUT % 2 == 0 else 512
    OCHUNKS = [(o, min(_ock, OUT - o)) for o in range(0, OUT, _ock)]
    # q,k each carry a WS factor from the pre-scaled weights
    scale = 1.0 / math.sqrt(dh) / (WS * WS)

    nc = bacc.Bacc("TRN2", target_bir_lowering=False, debug=False, num_devices=N_CORES)

    # ---- DRAM I/O ----
    # QK path: fp8 chunk-pair layout [128, pair-of-chunks, 2, .]
    xqT_d = nc.dram_tensor("xqT", [128, EP, 2, T], F8, kind="ExternalInput").ap()
    xkT_d = nc.dram_tensor("xkT", [128, EP, 2, T], F8, kind="ExternalInput").ap()
    wq_d = nc.dram_tensor("wq", [128, EP, 2, FEAT], F8, kind="ExternalInput").ap()
    wk_d = nc.dram_tensor("wk", [128, EP, 2, FEAT], F8, kind="ExternalInput").ap()
    bq_d = nc.dram_tensor("bq", [1, FEAT], BF16, kind="ExternalInput").ap()
    # V / out-proj path: bf16, unchanged layouts
    xvT_d = nc.dram_tensor("xvT", [EMB, T], BF16, kind="ExternalInput").ap()
    wv_d = nc.dram_tensor("wv", [EMB, FEAT], BF16, kind="ExternalInput").ap()
    bv_d = nc.dram_tensor("bv", [1, FEAT], BF16, kind="ExternalInput").ap()
    wo_d = nc.dram_tensor("wo", [FEAT, OUT], BF16, kind="ExternalInput").ap()
    out_d = nc.dram_tensor("out", [T, OUT], F32, kind="ExternalOutput").ap()

    with tile.TileContext(nc) as tc, ExitStack() as ctx:
        persist = ctx.enter_context(tc.tile_pool(name="persist", bufs=1))

        # ---- persistent SBUF tensors ----
        wq_sb = [persist.tile([128, 2, FEAT], F8, tag=f"wq{j}", name=f"wq{j}") for j in range(EP)]
        wk_sb = [persist.tile([128, 2, FEAT], F8, tag=f"wk{j}", name=f"wk{j}") for j in range(EP)]
        wv_sb = [persist.tile([128, FEAT], BF16, tag=f"wv{j}", name=f"wv{j}") for j in range(EC)]
        wo_sb = [persist.tile([128, OUT], BF16, tag=f"wo{j}", name=f"wo{j}") for j in range(FT)]
        bq_sb = persist.tile([1, FEAT], BF16, tag="bq", name="bq")
        bv_sb = persist.tile([1, FEAT], BF16, tag="bv", name="bv")
        ones_row = persist.tile([1, T], BF16, tag="ones_row", name="ones_row")
        # f32 base tile for Pool-engine exp: pow(e^scale, s) == exp(s*scale)
        base_sb = persist.tile([128, T2], F32, tag="base", name="base")
        xqT_sb = [persist.tile([128, 2, T], F8, tag=f"xq{j}", name=f"xq{j}") for j in range(EP)]
        xkT_sb = [persist.tile([128, 2, T], F8, tag=f"xk{j}", name=f"xk{j}") for j in range(EP)]
        xvT_sb = [persist.tile([128, T], BF16, tag=f"xv{j}", name=f"xv{j}") for j in range(EC)]
        # qT/kT: fp8, slot 0 = data, slot 1 = zeros (DoubleRow zero-slot)
        qT_sb = [persist.tile([128, 2, T], F8, tag=f"qT{j}", name=f"qT{j}") for j in range(FT)]
        kT_sb = [persist.tile([128, 2, T], F8, tag=f"kT{j}", name=f"kT{j}") for j in range(FT)]
        # V token-major, each head augmented with 64 ones columns so the PV
        # matmul emits the softmax denominator replicated on partitions 64-127
        v_sb = [persist.tile([128, NH * (dh + 64)], BF16, tag=f"v{i}", name=f"v{i}") for i in range(TT)]
        cn_sb = [persist.tile([128, T], BF16, tag=f"cn{j}", name=f"cn{j}") for j in range(FT)]

        # ---- weight/bias/x loads (Q/K path first: it gates head 0) ----
        # Loads alternate between the SP and ACT HWDGE queues so the two DMA
        # rings drain the startup burst in parallel (the ACT ALU is not
        # involved in its queue's transfers).
        _dmaq = [nc.sync, nc.scalar]

        def load(i, dst, src):
            _dmaq[i % 2].dma_start(dst, src)

        load(0, bq_sb[:], bq_d[:])
        for j in range(EP):
            load(j, wq_sb[j][:], wq_d[:, j, :, :])
            load(j + 1, xqT_sb[j][:], xqT_d[:, j, :, :])
            load(j, wk_sb[j][:], wk_d[:, j, :, :])
            load(j + 1, xkT_sb[j][:], xkT_d[:, j, :, :])
        load(0, bv_sb[:], bv_d[:])
        for j in range(EC):
            load(j, wv_sb[j][:], wv_d[j * 128 : (j + 1) * 128, :])
            load(j + 1, xvT_sb[j][:], xvT_d[j * 128 : (j + 1) * 128, :])
        for j in range(FT):
            load(j, wo_sb[j][:], wo_d[j * 128 : (j + 1) * 128, :])
        nc.vector.memset(ones_row[:], 1.0)
        nc.gpsimd.memset(base_sb[:], float(math.exp(scale)))
        # DoubleRow zero slots (one zeroed operand side makes the pair slot a
        # no-op; both sides zeroed to keep garbage NaN encodings out of the
        # PE). Pool runs these serially, so order by first use: feature tile
        # 0 gates the first scores, the V ones-columns gate head 0's first
        # context matmul, and feature tiles 1-2 are needed much later.
        nc.gpsimd.memset(qT_sb[0][:, 1, :], 0.0)
        nc.gpsimd.memset(kT_sb[0][:, 1, :], 0.0)
        # ones columns of augmented V (written once)
        for i in range(TT):
            vview = v_sb[i][:].rearrange("p (h x) -> p h x", x=dh + 64)
            nc.gpsimd.memset(vview[:, :, dh:], 1.0)
        for j in range(1, FT):
            nc.gpsimd.memset(qT_sb[j][:, 1, :], 0.0)
            nc.gpsimd.memset(kT_sb[j][:, 1, :], 0.0)

        # ---- compute: projections + attention + out-projection ----
        # PSUM budget (16KB/partition): ST ring 3 x [128,1024] f32 (12KB) +
        # ctx [128,1024] (4KB). There is NO separate projection psum: proj /
        # out-proj groups borrow ST ring slots (same tag, sliced), so the
        # score pipeline gets depth 3 - deep enough that the ACT exp stream
        # never stalls on slot recycling even with drip thunks in the PE
        # stream. PE instruction order is software-pipelined by hand.
        with (
            tc.tile_pool(name="stpsum", bufs=3, space="PSUM") as stpool,
            tc.tile_pool(name="ctpsum", bufs=1, space="PSUM") as ctpool,
            tc.tile_pool(name="ptpool", bufs=5) as ptpool,
            tc.tile_pool(name="stgpool", bufs=3) as stgpool,
            tc.tile_pool(name="normpool", bufs=3) as npool,
            tc.tile_pool(name="outsb", bufs=4) as osbpool,
        ):

            def proj_qk_group(j, t, n):
                qk = (
                    (wq_sb, bq_sb, xqT_sb, qT_sb),
                    (wk_sb, None, xkT_sb, kT_sb),
                )
                w_sb, b_sb, x_sb, dst = qk[t]
                ps = stpool.tile([128, T2], F32, tag="st", name="st")[:, 0:SCH]
                # q gets its bias via a K=1 rank-1 update; k needs none
                # (the key bias shifts all logits of a query equally and
                # softmax cancels it)
                if b_sb is not None:
                    nc.tensor.matmul(
                        ps[:],
                        b_sb[:, j * 128 : (j + 1) * 128],
                        ones_row[:, 0:SCH],
                        start=True,
                        stop=False,
                    )
                for e in range(EP):
                    nc.tensor.matmul(
                        ps[:],
                        w_sb[e][:, :, j * 128 : (j + 1) * 128],
                        x_sb[e][:, :, n * SCH : (n + 1) * SCH],
                        start=(b_sb is None and e == 0),
                        stop=(e == EP - 1),
                        perf_mode=DRMODE,
                    )
                nc.vector.tensor_copy(dst[j][:, 0, n * SCH : (n + 1) * SCH], ps[:])

            def proj_qk(j):
                # order q(n=0,1), k(n=0..), q(n=2,3): the first score tile of
                # head 0 needs only q chunks 0-1 and k chunk 0, so this
                # unblocks the exp stream ~3us earlier during the load phase
                order = [(0, n) for n in range(min(2, NSCH))]
                order += [(1, n) for n in range(NSCH)]
                order += [(0, n) for n in range(min(2, NSCH), NSCH)]
                for t, n in order:
                    proj_qk_group(j, t, n)

            def proj_v(tiles=None):
                for i in tiles if tiles is not None else range(TT):
                    ps = stpool.tile([128, T2], F32, tag="st", name="st")[:, 0:FEAT]
                    nc.tensor.matmul(
                        ps[:], ones_row[:, 0:128], bv_sb[:], start=True, stop=False
                    )
                    for e in range(EC):
                        nc.tensor.matmul(
                            ps[:],
                            xvT_sb[e][:, i * 128 : (i + 1) * 128],
                            wv_sb[e][:],
                            start=False,
                            stop=(e == EC - 1),
                        )
                    dst = v_sb[i][:].rearrange("p (h x) -> p h x", x=dh + 64)[:, :, 0:dh]
                    srcv = ps[:].rearrange("p (h d) -> p h d", d=dh)
                    # alternate DVE/ACT: these copies run inside head 0 where
                    # PE carries the V projections and ACT has idle slots
                    # (DVE first: an ACT copy of tile 0 would sit ahead of
                    # exp(0) in ACT's queue and delay the whole exp stream)
                    if i % 2 == 1:
                        nc.scalar.copy(dst, srcv)
                    else:
                        nc.vector.tensor_copy(dst, srcv)

            def st_tile(i, kT_h, qT_h, s0):
                st = stpool.tile([128, T2], F32, tag="st", name="st")
                for n in range(NSCH2):
                    nc.tensor.matmul(
                        st[:, n * SCH2 : (n + 1) * SCH2],
                        kT_h[:, :, i * 128 : (i + 1) * 128],
                        qT_h[:, :, s0 + n * SCH2 : s0 + (n + 1) * SCH2],
                        start=True,
                        stop=True,
                        perf_mode=DRMODE,
                    )
                return st

            pending_sts = []

            def head_args(h, sh):
                ft, half = h // 2, (h % 2) * 64
                return (
                    kT_sb[ft][half : half + 64, :, :],
                    qT_sb[ft][half : half + 64, :, :],
                    sh * T2,
                )

            def head(h, sh, filler=None, nxt=None):
                # keeps 2 score tiles in flight and pre-issues the NEXT
                # head's first 2 before this head's last context matmul, so
                # the ScalarE exp stream never stalls at head boundaries
                ft, half = h // 2, (h % 2) * 64
                kT_h, qT_h, s0 = head_args(h, sh)
                ct = ctpool.tile([128, T2], F32, tag="ct", name="ct")
                sts = pending_sts[:]
                del pending_sts[:]
                while len(sts) < min(2, TT):
                    sts.append(st_tile(len(sts), kT_h, qT_h, s0))
                nissued = 0
                # PV accumulation over key tiles is order-independent: the
                # Pool-exp'd tiles of each group of 4 are accumulated after
                # their group's ACT tiles so the longer DVE-copy + Pool-pow
                # chain gets extra PE steps of slack.
                staged_set = set(staged) if TT >= 8 else set()
                pv_order = []
                for g0 in range(0, TT, 4):
                    grp = list(range(g0, min(g0 + 4, TT)))
                    pv_order += [i for i in grp if i not in staged_set]
                    pv_order += [i for i in grp if i in staged_set]
                first_pv, last_pv = pv_order[0], pv_order[-1]
                emit_after = {i: [] for i in range(TT)}
                _ptr = [0]
                for i in range(TT):
                    while _ptr[0] < len(pv_order) and pv_order[_ptr[0]] <= i:
                        emit_after[i].append(pv_order[_ptr[0]])
                        _ptr[0] += 1

                def pv(i, pt):
                    for n in range(NSCH2):
                        nc.tensor.matmul(
                            ct[:, n * SCH2 : (n + 1) * SCH2],
                            v_sb[i][:, h * (dh + 64) : (h + 1) * (dh + 64)],
                            pt[:, n * SCH2 : (n + 1) * SCH2],
                            start=(i == first_pv),
                            stop=(i == last_pv),
                        )

                pts = {}
                for i in range(TT):
                    st = sts.pop(0)
                    pt = ptpool.tile([128, T2], BF16, tag="pt", name="pt")
                    pts[i] = pt
                    if i in staged_set:
                        # staged tile: DVE copies psum->sbuf bf16, Pool exps
                        # it via pow (Pool cannot read PSUM directly);
                        # chunked 512-wide so the chain pipelines against the
                        # score matmuls instead of serializing behind them
                        stg = stgpool.tile([128, T2], BF16, tag="stg", name="stg")
                        for n in range(NSCH2):
                            csl = slice(n * SCH2, (n + 1) * SCH2)
                            nc.vector.tensor_copy(stg[:, csl], st[:, csl])
                            nc.gpsimd.tensor_tensor(
                                pt[:, csl], base_sb[:, csl], stg[:, csl],
                                op=mybir.AluOpType.pow,
                            )
                    else:
                        nc.scalar.activation(
                            pt[:], st[:], mybir.ActivationFunctionType.Exp, scale=scale
                        )
                    if i + 2 < TT:
                        sts.append(st_tile(i + 2, kT_h, qT_h, s0))
                    elif nxt is not None and nissued < min(2, TT):
                        pending_sts.append(st_tile(nissued, *head_args(*nxt)))
                        nissued += 1
                    if filler is not None:
                        filler(i)
                    for j in emit_after[i]:
                        pv(j, pts.pop(j))

                # normalize: cn[f, s] = ct[f, s] * (1 / ct[64.., s]);
                # chunked so the ct psum frees in halves and the next head's
                # first context matmul can start after chunk 0 is drained
                recip = npool.tile([64, T2], F32, tag="recip", name="recip")
                for n in range(NSCH2):
                    csl = slice(n * SCH2, (n + 1) * SCH2)
                    nc.vector.reciprocal(recip[:, csl], ct[64:128, csl])
                    nc.vector.tensor_tensor(
                        cn_sb[ft][half : half + 64, s0 + n * SCH2 : s0 + (n + 1) * SCH2],
                        ct[0:64, csl],
                        recip[:, csl],
                        op=mybir.AluOpType.mult,
                    )

            osb_state = {}

            def outproj_chunk(i, ci, use_act=False):
                oc, ow = OCHUNKS[ci]
                if i not in osb_state:
                    osb_state[i] = osbpool.tile([128, OUT], F32, tag="osb", name="osb")
                osb = osb_state[i]
                ps = stpool.tile([128, T2], F32, tag="st", name="st")[:, 0:ow]
                for f in range(FT):
                    nc.tensor.matmul(
                        ps[:],
                        cn_sb[f][:, i * 128 : (i + 1) * 128],
                        wo_sb[f][:, oc : oc + ow],
                        start=(f == 0),
                        stop=(f == FT - 1),
                    )
                if use_act:
                    nc.scalar.copy(osb[:, oc : oc + ow], ps[:])
                else:
                    nc.vector.tensor_copy(osb[:, oc : oc + ow], ps[:])
                if ci == len(OCHUNKS) - 1:
                    nc.sync.dma_start(out_d[i * 128 : (i + 1) * 128, :], osb[:])
                    del osb_state[i]

            def outproj(tiles):
                # final flush: no exp stream left, so the psum-drain copies
                # alternate ACT/DVE to keep pace with the PE chunks
                for i in tiles:
                    for ci in range(len(OCHUNKS)):
                        outproj_chunk(i, ci, use_act=(i + ci) % 2 == 0)

            proj_qk(0)
            # pre-issue head 0's first score tiles BEFORE any V work: V
            # depends on the last-arriving xvT DMAs and must not gate exp_0
            for z in range(min(2, TT)):
                pending_sts.append(st_tile(z, *head_args(0, 0)))
            # V tile i is first needed at head 0's CT step i: emit tile 0/1
            # up front and drip the rest into head 0's pipeline
            proj_v(range(2))

            def v_filler(i):
                if i + 2 < TT:
                    proj_v([i + 2])

            # ACT's exp stream is the co-pacer with PE: any multi-us PE-only
            # burst starves it (no new score tiles while it runs). All
            # deferrable PE work is therefore queued as ~0.5us thunks and
            # dripped one per odd pipeline step inside the head loops.
            drip = []

            def drip_filler(i):
                if i % 2 == 1 and drip:
                    drip.pop(0)()

            half_tiles = T2 // 128 if NSH == 2 else 0
            seq = [
                (2 * p + z, sh)
                for p in range(NH // 2)
                for sh in range(NSH)
                for z in (0, 1)
            ]
            if NSH == 2:
                for pos, (h, sh) in enumerate(seq):
                    if h % 2 == 0 and sh == 0 and h // 2 + 1 < FT:
                        # queue next feature-tile's projections at pair start;
                        # they drain over the pair's ~24 drip slots
                        for t in range(2):
                            for n in range(NSCH):
                                drip.append(
                                    lambda j=h // 2 + 1, t=t, n=n: proj_qk_group(j, t, n)
                                )
                    nxt = seq[pos + 1] if pos + 1 < len(seq) else None
                    filler = v_filler if (h, sh) == (0, 0) else drip_filler
                    head(h, sh, filler, nxt=nxt)
                    if h == NH - 1 and sh == 0:
                        # queries [0, T2) have every head's context: their
                        # out-proj chunks drip into the remaining half-heads
                        for i in range(half_tiles):
                            for ci in range(len(OCHUNKS)):
                                drip.append(lambda i=i, ci=ci: outproj_chunk(i, ci))
                while drip:
                    drip.pop(0)()
                outproj(range(half_tiles, TT))
            else:
                pos = 0
                for p in range(NH // 2):
                    for sh in range(NSH):
                        nxt = seq[pos + 1] if pos + 1 < len(seq) else None
                        head(2 * p, sh, v_filler if (p, sh) == (0, 0) else None, nxt=nxt)
                        pos += 1
                        nxt = seq[pos + 1] if pos + 1 < len(seq) else None
                        head(2 * p + 1, sh, nxt=nxt)
                        pos += 1
                        if p + 1 < FT:
                            proj_qk(p + 1)
                outproj(range(TT))

    nc.compile()
    return nc


def _pair_pack_f8(a):
    """[E, N] -> [128, E//256, 2, N] fp8 chunk-pair layout."""
    e, n = a.shape
    return np.ascontiguousarray(
        a.reshape(e // 256, 2, 128, n).transpose(2, 0, 1, 3)
    ).astype(NP_F8)


def shard_inputs(query, key, value, wq, bq, wk, bk, wv, bv, wo):
    """Build the 8 per-core input maps (host-side cast/transpose/slice).

    bk is accepted for signature compatibility but unused: the key bias adds
    a per-query constant to every logit, which softmax cancels exactly.
    """
    in_maps = []
    xT = {}
    for b in range(B):
        xT[b] = (
            _pair_pack_f8(query[b].T),
            _pair_pack_f8(key[b].T),
            np.ascontiguousarray(value[b].T).astype(NP_BF16),
        )
    gw = {}
    for g in range(2):
        hs = slice(g * G, (g + 1) * G)
        gw[g] = dict(
            wq=_pair_pack_f8(wq[hs].transpose(1, 0, 2).reshape(E, G * DH) * WS),
            wk=_pair_pack_f8(wk[hs].transpose(1, 0, 2).reshape(E, G * DH) * WS),
            wv=np.ascontiguousarray(wv[hs].transpose(1, 0, 2).reshape(E, G * DH)).astype(NP_BF16),
            wo=np.ascontiguousarray(wo[g * G * DH : (g + 1) * G * DH, :]).astype(NP_BF16),
            bq=np.ascontiguousarray(bq[hs].reshape(1, G * DH) * WS).astype(NP_BF16),
            bv=np.ascontiguousarray(bv[hs].reshape(1, G * DH)).astype(NP_BF16),
        )
    for c in range(N_CORES):
        b, g = c // 2, c % 2
        m = dict(xqT=xT[b][0], xkT=xT[b][1], xvT=xT[b][2])
        m.update(gw[g])
        in_maps.append(m)
    return in_maps


_CACHED_NC = None


def kernel(query, key, value, wq, bq, wk, bk, wv, bv, wo, bo):
    global _CACHED_NC
    query, key, value = (np.asarray(a, np.float32) for a in (query, key, value))
    wq, bq, wk, bk, wv, bv, wo, bo = (
        np.asarray(a, np.float32) for a in (wq, bq, wk, bk, wv, bv, wo, bo)
    )
    in_maps = shard_inputs(query, key, value, wq, bq, wk, bk, wv, bv, wo)
    if _CACHED_NC is None:
        _CACHED_NC = build_nc()
    res = run_bass_kernel_spmd(_CACHED_NC, in_maps, list(range(N_CORES)))
    out = np.empty((B, S, E), np.float32)
    for b in range(B):
        out[b] = res.results[2 * b]["out"] + res.results[2 * b + 1]["out"] + bo[None, :]
    return out
